# revision 1
# baseline (speedup 1.0000x reference)
"""LCGP prediction kernel for Trainium2, sharded over 8 NeuronCores.

Strategy (expert-parallel over the q=8 GP components, one per core):
  Per core q, the n0=2048 test axis is processed in 4 pipelined quarters:
    phase 1: C0T[n, m] = exp(lLmb0[q] + ln(S) - ||a_m - b_n||^2) via an fp8e4
        DoubleRow PE matmul over hi/lo-split fp8 feature-row pairs (40 virtual
        contraction rows; ~8-bit products, 3-level sq rows). ACT exp emits the
        scaled C0T in fp16 (c16), DVE down-converts to fp8e4 (c8).
    ghat[m]  = C0T.T @ CinvM[q] from the fp16 c16 (precision-critical path).
    phase 2: t = C0 @ Th[q] as an fp8e4 DoubleRow GEMM (256-deep contraction
        per matmul, 0.5 cyc/col); Th pre-scaled/converted to fp8 on host and
        streamed per quarter. sumt2[m] = sum_r t[m,r]^2 via ACT Square with
        accum_out per [128,512] PSUM tile; raw per-r sums reduced on host.
  Only quarter 0's exp production is PE-exposed (kk-major chase batch fills
  it); later quarters' phase 1 hides as sprinkles inside the previous
  quarter's GEMM, keeping every engine's priority stream temporally feasible.
  Host: tiny [q,n0] -> [p,n0] psi projection in fp32 numpy.

fp8 error budget (validated in sim + device): sumt2 averages quantization
noise over the r-contraction (~2e-3); ghat stays fp16 after exp (~9e-4).
"""

import os

import numpy as np
import ml_dtypes

import concourse.bacc as bacc
import concourse.bass as bass
import concourse.mybir as mybir
import concourse.tile as tile

P = 128
FP32 = mybir.dt.float32
FP16 = mybir.dt.float16
FP8 = mybir.dt.float8e4
F8NP = ml_dtypes.float8_e4m3

# Full-size problem dims (hardcoded per spec: q=8, d=8, p=64, n=4096, n0=2048)
Q_FULL = 8
N_FULL = 4096
N0_FULL = 2048

S_C0 = np.float32(32.0)          # C0 pre-scale folded into the exp bias
LN_S_C0 = float(np.log(S_C0))


def build_nc(n=N_FULL, n0=N0_FULL, rb=512, mh=512, fk=20, mc=512, debug=False):
    """Build the single-core Bass program (same program on all 8 cores)."""
    kt = n // P            # 32 contraction k-tiles of 128
    kt2 = kt // 2          # 16 DoubleRow k-steps of 256
    nrb = n // rb          # 8 r-blocks of the big GEMM
    nh = n0 // mh          # 2 m-halves
    mt = mh // P           # 8 m-tiles per half
    nmc = mh // mc         # 2 phase-1 chunks per half

    nc = bacc.Bacc("TRN2", target_bir_lowering=False, debug=debug)

    a_feat = nc.dram_tensor("a_feat", [fk, 2, n0], FP8, kind="ExternalInput")
    b_feat = nc.dram_tensor("b_feat", [fk, 2, n], FP8, kind="ExternalInput")
    th8 = nc.dram_tensor("th8", [P, kt2, 2, n], FP8, kind="ExternalInput")
    cinv = nc.dram_tensor("cinv", [P, kt], FP16, kind="ExternalInput")
    ghat_o = nc.dram_tensor("ghat", [n0 // P, P], FP32, kind="ExternalOutput")
    # raw per-r-block square sums; host reduces the last axis
    sumt2_o = nc.dram_tensor("sumt2", [n0 // P, P, nrb], FP32,
                             kind="ExternalOutput")

    with tile.TileContext(nc) as tc:
        with (
            tc.tile_pool(name="feat", bufs=1) as featp,
            tc.tile_pool(name="c16", bufs=1) as c16p,
            tc.tile_pool(name="c8", bufs=2) as c8p,
            tc.tile_pool(name="slab", bufs=3 * kt2) as slabp,
            tc.tile_pool(name="scr", bufs=4) as scrp,
            tc.tile_pool(name="gsb", bufs=2 * mt + 4) as gsbp,
            tc.tile_pool(name="sqps", bufs=2, space=bass.MemorySpace.PSUM) as sqpsp,
            tc.tile_pool(name="tps", bufs=5, space=bass.MemorySpace.PSUM) as tpsp,
            tc.tile_pool(name="gps", bufs=1, space=bass.MemorySpace.PSUM) as gpsp,
        ):
            bf = featp.tile([fk, 2, n], FP8, tag="bf")
            af = featp.tile([fk, 2, n0], FP8, tag="af")
            cv = featp.tile([P, kt], FP16, tag="cv")
            # head pieces first for an early phase-1 start, then one bulk
            # DMA each, so r0's slab DMAs aren't stuck behind input pieces
            nc.sync.dma_start(bf[:, :, 0:mc], b_feat[:, :, 0:mc])
            nc.gpsimd.dma_start(af[:, :, 0:mc], a_feat[:, :, 0:mc])
            nc.sync.dma_start(bf[:, :, mc:n], b_feat[:, :, mc:n])
            nc.gpsimd.dma_start(af[:, :, mc:n0], a_feat[:, :, mc:n0])
            nc.gpsimd.dma_start(cv[:], cinv[:])

            def p1_chunk(moff, c16, c8, j, lo, w):
                # columns [lo, lo+w) of this slice (global m = moff+lo..)
                ps = sqpsp.tile([P, mc], FP32, tag="sqps")
                nc.tensor.matmul(
                    ps[:, 0:w],
                    bf[:, :, j * P:(j + 1) * P],
                    af[:, :, moff + lo: moff + lo + w],
                    start=True, stop=True,
                    perf_mode=mybir.MatmulPerfMode.DoubleRow,
                )
                nc.scalar.activation(
                    c16[:, j, lo:lo + w], ps[:, 0:w],
                    mybir.ActivationFunctionType.Exp,
                    bias=0.0, scale=-1.0,
                )
                nc.vector.tensor_copy(
                    c8[:, j, lo:lo + w], c16[:, j, lo:lo + w],
                )

            def ghat_group(h, c16, i):
                gp = gpsp.tile([P, 1], FP32, tag="gps", name=f"gp_{h}_{i}")
                for j in range(kt):
                    nc.tensor.matmul(
                        gp[:], c16[:, j, i * P:(i + 1) * P], cv[:, j:j + 1],
                        start=(j == 0), stop=(j == kt - 1),
                        skip_group_check=True,
                    )
                gh = gsbp.tile([P, 1], FP32, tag="ghsb")
                nc.vector.tensor_copy(gh[:], gp[:])
                nc.sync.dma_start(ghat_o[h * mt + i, :], gh[:])

            def slab_dmas(r):
                slabs = []
                for kk in range(kt2):
                    sl = slabp.tile([P, 2, rb], FP8, tag="slab",
                                    name=f"sl_{r}_{kk}")
                    eng = nc.sync if kk % 2 == 0 else nc.gpsimd
                    eng.dma_start(
                        sl[:], th8[:, kk, :, r * rb:(r + 1) * rb])
                    slabs.append(sl)
                return slabs

            def dr_mm(c8, tp, slabs, i, kk):
                nc.tensor.matmul(
                    tp[:],
                    c8[:, 2 * kk:2 * kk + 2, i * P:(i + 1) * P],
                    slabs[kk][:],
                    start=(kk == 0), stop=(kk == kt2 - 1),
                    perf_mode=mybir.MatmulPerfMode.DoubleRow,
                    skip_group_check=True,
                )

            def square(h, gaccs, tp, r, i):
                sc = scrp.tile([P, rb], FP16, tag="scr",
                               name=f"sc_{h}_{r}_{i}")
                nc.scalar.activation(
                    sc[:], tp[:], mybir.ActivationFunctionType.Square,
                    accum_out=gaccs[i][:, r:r + 1],
                )

            # ---- emission order == scheduler priority (keep each engine's
            # priority stream temporally feasible: the wait queue is shallow,
            # so a long run of not-yet-ready instructions stalls the engine).
            # n0 is processed in progressive slices: only the narrow first
            # slice's exp production is PE-exposed; each later slice's
            # phase 1 hides as sprinkles inside the previous slice's GEMM.

            SL = (256, 512, 640, 640)          # slice widths, sum = n0
            nsl = len(SL)
            moffs = [sum(SL[:s]) for s in range(nsl)]
            c16s = [None] * nsl
            c8s = [None] * nsl

            def chunks_of(w):
                if w <= mc:
                    return [(0, w)]
                h = w // 2
                return [(0, h), (h, w - h)]

            def alloc_slice(s):
                c16s[s] = c16p.tile([P, kt, SL[s]], FP16, tag="c16",
                                    name=f"c16_{s}")
                c8s[s] = c8p.tile([P, kt, SL[s]], FP8, tag="c8",
                                  name=f"c8_{s}")

            def ghat_group_sl(s, i):
                gp = gpsp.tile([P, 1], FP32, tag="gps", name=f"gp_{s}_{i}")
                for j in range(kt):
                    nc.tensor.matmul(
                        gp[:], c16s[s][:, j, i * P:(i + 1) * P],
                        cv[:, j:j + 1],
                        start=(j == 0), stop=(j == kt - 1),
                        skip_group_check=True,
                    )
                gh = gsbp.tile([P, 1], FP32, tag="ghsb")
                nc.vector.tensor_copy(gh[:], gp[:])
                nc.sync.dma_start(ghat_o[moffs[s] // P + i, :], gh[:])

            slabs0 = slab_dmas(0)
            # phase 1 slice 0 (chain paced by ACT exp): two j-planes share
            # one PSUM bank so each exp/convert covers 512 columns.
            alloc_slice(0)
            w0 = SL[0]
            for jp in range(kt // 2):
                ps = sqpsp.tile([P, mc], FP32, tag="sqps")
                for u in range(2):
                    nc.tensor.matmul(
                        ps[:, u * w0:(u + 1) * w0],
                        bf[:, :, (2 * jp + u) * P:(2 * jp + u + 1) * P],
                        af[:, :, moffs[0]:moffs[0] + w0],
                        start=True, stop=True,
                        perf_mode=mybir.MatmulPerfMode.DoubleRow,
                    )
                nc.scalar.activation(
                    c16s[0][:, 2 * jp:2 * jp + 2, :], ps[:],
                    mybir.ActivationFunctionType.Exp,
                    bias=0.0, scale=-1.0,
                )
                nc.vector.tensor_copy(
                    c8s[0][:, 2 * jp:2 * jp + 2, :],
                    c16s[0][:, 2 * jp:2 * jp + 2, :],
                )

            # slice-0 GEMM r0: kk-major across its m-tiles (chase fill; the
            # spare tps slots chase r1+ groups via pool rotation)
            mt0 = SL[0] // P
            gaccs0 = [gsbp.tile([P, nrb], FP32, tag="gacc",
                                name=f"gacc_0_{i}") for i in range(mt0)]
            tA = {i: tpsp.tile([P, rb], FP32, tag="tps", name=f"tp_a_{i}")
                  for i in range(mt0)}
            for kk in range(kt2):
                for i in range(mt0):
                    dr_mm(c8s[0], tA[i], slabs0, i, kk)
            for i in range(mt0):
                square(0, gaccs0, tA[i], 0, i)

            def emit_gemm_sprinkled(s, gaccs_pre):
                """GEMM for slice s; sprinkle phase-1 of slice s+1."""
                c8 = c8s[s]
                mts = SL[s] // P
                if gaccs_pre is None:
                    gaccs = [gsbp.tile([P, nrb], FP32, tag="gacc",
                                       name=f"gacc_{s}_{i}")
                             for i in range(mts)]
                else:
                    gaccs = gaccs_pre
                ns = s + 1 if s + 1 < nsl else None
                if ns is not None:
                    alloc_slice(ns)
                    p1_list = [(j, lo, w) for j in range(kt)
                               for lo, w in chunks_of(SL[ns])]
                r_lo = 1 if s == 0 else 0
                ngrp = (nrb - r_lo) * mts
                g = 0
                cc = 0
                for r in range(r_lo, nrb):
                    slabs = slab_dmas(r)
                    for i in range(mts):
                        tp = tpsp.tile([P, rb], FP32, tag="tps",
                                       name=f"tp_{s}_{r}_{i}")
                        for kk in range(kt2):
                            dr_mm(c8, tp, slabs, i, kk)
                        square(s, gaccs, tp, r, i)
                        g += 1
                        if r == r_lo:
                            ghat_group_sl(s, i)
                        if ns is not None:
                            want = g * len(p1_list) // ngrp
                            while cc < want:
                                p1_chunk(moffs[ns], c16s[ns], c8s[ns],
                                         *p1_list[cc])
                                cc += 1
                if ns is not None:
                    while cc < len(p1_list):
                        p1_chunk(moffs[ns], c16s[ns], c8s[ns], *p1_list[cc])
                        cc += 1
                for i in range(mts):
                    nc.sync.dma_start(sumt2_o[moffs[s] // P + i], gaccs[i][:])

            emit_gemm_sprinkled(0, gaccs0)
            for s in range(1, nsl):
                emit_gemm_sprinkled(s, None)

    nc.compile()
    return nc


def _features_for_q(x0s, x, inv_l_q, lLmb0_q, fk=20):
    """Host prep: hi/lo-split fp8e4 feature row pairs (DoubleRow layout) so
    the PE computes sq_mod[n, m] = ||a_m - b_n||^2 - lLmb0 - ln(S_C0) with
    ~8-bit products and 3-level sq rows (abs err ~2e-3)."""
    f32 = np.float32

    def hilo8(v, levels=2):
        outs = []
        r = v.astype(f32).copy()
        for _ in range(levels):
            h = r.astype(F8NP)
            outs.append(h)
            r = r - h.astype(f32)
        return outs

    a = (x0s * inv_l_q).astype(f32)            # [n0, d]
    b = (x * inv_l_q).astype(f32)              # [n, d]
    sqa = (a * a).sum(-1, dtype=f32) - f32(lLmb0_q) - f32(LN_S_C0)
    sqb = (b * b).sum(-1, dtype=f32)
    m2a = (-2.0 * a).astype(f32)
    Ah, Al = hilo8(m2a)
    Bh, Bl = hilo8(b)
    sqa3 = hilo8(sqa, 3)
    sqb3 = hilo8(sqb, 3)
    d = a.shape[1]
    n0, n = a.shape[0], b.shape[0]
    ones_a = np.ones(n0, F8NP)
    ones_b = np.ones(n, F8NP)
    pairs = []                                  # (a_row [n0], b_row [n])
    for dd in range(d):
        pairs.append((Ah[:, dd], Bh[:, dd]))
        pairs.append((Ah[:, dd], Bl[:, dd]))
        pairs.append((Al[:, dd], Bh[:, dd]))
        pairs.append((Al[:, dd], Bl[:, dd]))
    for lev in sqa3:
        pairs.append((lev, ones_b))
    for lev in sqb3:
        pairs.append((ones_a, lev))
    assert len(pairs) <= 2 * fk
    af = np.zeros((fk, 2, n0), F8NP)
    bf = np.zeros((fk, 2, n), F8NP)
    for v, (ar, br) in enumerate(pairs):
        af[v // 2, v % 2] = ar
        bf[v // 2, v % 2] = br
    return af, bf


def _th_scale(th_q):
    """Power-of-two scale putting max |Th| into (96, 192] for fp8e4."""
    m = float(np.abs(th_q).max())
    if m == 0.0:
        return np.float32(1.0)
    return np.float32(2.0 ** np.floor(np.log2(192.0 / m)))


def prep_core_inputs(inputs, q, fk=32):
    """Per-core (per-component) input map for the device kernel."""
    f16, f32 = np.float16, np.float32
    x0 = np.asarray(inputs["x0"], f32)
    x = np.asarray(inputs["x"], f32)
    x_min = np.asarray(inputs["x_min"], f32)
    x_max = np.asarray(inputs["x_max"], f32)
    lLmb = np.asarray(inputs["lLmb"], f32)
    lLmb0 = np.asarray(inputs["lLmb0"], f32)
    x0s = (x0 - x_min) / (x_max - x_min)
    inv_l = np.exp(-0.5 * lLmb[q]).astype(f32)
    af, bf = _features_for_q(x0s, x, inv_l, lLmb0[q], fk=20)
    cinv = np.asarray(inputs["CinvM"], f32)[q].astype(f16)
    n = cinv.shape[0]
    cinv_t = np.ascontiguousarray(cinv.reshape(n // P, P).T)   # [128, kt]
    th_q = np.asarray(inputs["Th"], f32)[q]
    s_th = _th_scale(th_q)
    # [128, kt2, 2, n]: th8[p, kk, i, c] = s_th * Th[kk*256 + i*128 + p, c]
    th8 = np.ascontiguousarray(
        (th_q.reshape(n // 256, 2, P, n) * s_th).transpose(2, 0, 1, 3)
    ).astype(F8NP)
    return {"a_feat": af, "b_feat": bf, "th8": th8, "cinv": cinv_t}


def finish_host(inputs, ghat_all, sumt2_all):
    """Final tiny [q,n0] -> [p,n0] projection, fp32 on host (mirrors reference)."""
    f32 = np.float32
    lLmb0 = np.asarray(inputs["lLmb0"], f32)
    lnug = np.asarray(inputs["lnugGPs"], f32)
    lsig = np.asarray(inputs["lsigma2s"], f32)
    phi = np.asarray(inputs["phi"], f32)
    ystd = np.asarray(inputs["ystd"], f32)
    ymean = np.asarray(inputs["ymean"], f32)

    c00 = (np.exp(lLmb0) * (1.0 + np.exp(lnug))).astype(f32)[:, None]
    gvar = c00 - sumt2_all                        # [q, n0]
    sig = np.exp(lsig).astype(f32)                # [p]
    psi = (phi * np.sqrt(sig)[:, None]).astype(f32)
    predmean = (psi @ ghat_all).astype(f32)       # [p, n0]
    confvar = (gvar.T @ (psi ** 2).T).astype(f32)  # [n0, p]
    predvar = confvar + sig
    ypred = (predmean * ystd + ymean).astype(f32)
    yconfvar = (confvar.T * ystd ** 2).astype(f32)
    ypredvar = (predvar.T * ystd ** 2).astype(f32)
    return ypred, ypredvar, yconfvar


_NC_CACHE = {}
LAST_RESULTS = None


def kernel(**inputs):
    from concourse.bass_utils import run_bass_kernel_spmd

    global LAST_RESULTS
    q_n = Q_FULL
    n0 = N0_FULL

    if "nc" not in _NC_CACHE:
        _NC_CACHE["nc"] = build_nc()
    nc = _NC_CACHE["nc"]

    th_f32 = np.asarray(inputs["Th"], np.float32)
    s_ths = [_th_scale(th_f32[q]) for q in range(q_n)]
    in_maps = [prep_core_inputs(inputs, q) for q in range(q_n)]
    core_ids = list(range(q_n))
    res = run_bass_kernel_spmd(
        nc, in_maps, core_ids,
        trace=bool(os.environ.get("LCGP_TRACE")),
    )
    LAST_RESULTS = res

    ghat_all = np.zeros((q_n, n0), np.float32)
    sumt2_all = np.zeros((q_n, n0), np.float32)
    for q in range(q_n):
        ghat_all[q] = np.asarray(res.results[q]["ghat"]).reshape(n0) / S_C0
        raw = np.asarray(res.results[q]["sumt2"], np.float32)  # [16, 128, nrp]
        sumt2_all[q] = raw.sum(-1, dtype=np.float32).reshape(n0) \
            / (S_C0 * s_ths[q]) ** 2

    return finish_host(inputs, ghat_all, sumt2_all)



# revision 45
# speedup vs baseline: 1.0159x; 1.0159x over previous
"""LCGP prediction kernel for Trainium2, sharded over 8 NeuronCores.

Strategy (expert-parallel over the q=8 GP components, one per core):
  Per core q, the n0=2048 test axis is processed in 4 pipelined quarters:
    phase 1: C0T[n, m] = exp(lLmb0[q] + ln(S) - ||a_m - b_n||^2) via an fp8e4
        DoubleRow PE matmul over hi/lo-split fp8 feature-row pairs (40 virtual
        contraction rows; ~8-bit products, 3-level sq rows). ACT exp emits the
        scaled C0T in fp16 (c16), DVE down-converts to fp8e4 (c8).
    ghat[m]  = C0T.T @ CinvM[q] from the fp16 c16 (precision-critical path).
    phase 2: t = C0 @ Th[q] as an fp8e4 DoubleRow GEMM (256-deep contraction
        per matmul, 0.5 cyc/col); Th pre-scaled/converted to fp8 on host and
        streamed per quarter. sumt2[m] = sum_r t[m,r]^2 via ACT Square with
        accum_out per [128,512] PSUM tile; raw per-r sums reduced on host.
  Only quarter 0's exp production is PE-exposed (kk-major chase batch fills
  it); later quarters' phase 1 hides as sprinkles inside the previous
  quarter's GEMM, keeping every engine's priority stream temporally feasible.
  Host: tiny [q,n0] -> [p,n0] psi projection in fp32 numpy.

fp8 error budget (validated in sim + device): sumt2 averages quantization
noise over the r-contraction (~2e-3); ghat stays fp16 after exp (~9e-4).
"""

import os

import numpy as np
import ml_dtypes

import concourse.bacc as bacc
import concourse.bass as bass
import concourse.mybir as mybir
import concourse.tile as tile

P = 128
FP32 = mybir.dt.float32
FP16 = mybir.dt.float16
FP8 = mybir.dt.float8e4
F8NP = ml_dtypes.float8_e4m3

# Full-size problem dims (hardcoded per spec: q=8, d=8, p=64, n=4096, n0=2048)
Q_FULL = 8
N_FULL = 4096
N0_FULL = 2048

S_C0 = np.float32(32.0)          # C0 pre-scale folded into the exp bias
LN_S_C0 = float(np.log(S_C0))


def build_nc(n=N_FULL, n0=N0_FULL, rb=512, mh=512, fk=20, mc=512, warm=10,
             lead=0, debug=False):
    """Build the single-core Bass program (same program on all 8 cores)."""
    kt = n // P            # 32 contraction k-tiles of 128
    kt2 = kt // 2          # 16 DoubleRow k-steps of 256
    nrb = n // rb          # 8 r-blocks of the big GEMM
    nh = n0 // mh          # 2 m-halves
    mt = mh // P           # 8 m-tiles per half
    nmc = mh // mc         # 2 phase-1 chunks per half

    nc = bacc.Bacc("TRN2", target_bir_lowering=False, debug=debug)

    a_feat = nc.dram_tensor("a_feat", [fk, 2, n0], FP8, kind="ExternalInput")
    b_feat = nc.dram_tensor("b_feat", [fk, 2, n], FP8, kind="ExternalInput")
    th8 = nc.dram_tensor("th8", [P, kt2, 2, n], FP8, kind="ExternalInput")
    cinv = nc.dram_tensor("cinv", [P, kt], FP16, kind="ExternalInput")
    ghat_o = nc.dram_tensor("ghat", [n0 // P, P], FP32, kind="ExternalOutput")
    # raw per-r-block square sums; host reduces the last axis. Column nrb
    # holds the final tile's extra split-chunk sum (others never write it).
    sumt2_o = nc.dram_tensor("sumt2", [n0 // P, P, nrb + 1], FP32,
                             kind="ExternalOutput")

    with tile.TileContext(nc) as tc:
        with (
            tc.tile_pool(name="feat", bufs=1) as featp,
            tc.tile_pool(name="c16", bufs=1) as c16p,
            tc.tile_pool(name="c8", bufs=2) as c8p,
            tc.tile_pool(name="slab", bufs=3 * kt2) as slabp,
            tc.tile_pool(name="scr", bufs=4) as scrp,
            tc.tile_pool(name="gsb", bufs=2 * mt + 4) as gsbp,
            tc.tile_pool(name="sqps", bufs=2, space=bass.MemorySpace.PSUM) as sqpsp,
            tc.tile_pool(name="tps", bufs=6, space=bass.MemorySpace.PSUM) as tpsp,
        ):
            bf = featp.tile([fk, 2, n], FP8, tag="bf")
            af = featp.tile([fk, 2, n0], FP8, tag="af")
            cv = featp.tile([P, kt], FP16, tag="cv")

            # ---- ACT exp-table preload: the first input DMA lands ~2.4us
            # in; preload the exp table (1283ns) behind that wait so the
            # first real exp doesn't pay it. (Square shares exp's table.)
            wsrc = featp.tile([P, 2, P], FP8, tag="wsrc")
            nc.vector.memset(wsrc[:], 0.0)
            wact = featp.tile([P, 8], FP16, tag="wact")
            nc.scalar.activation(
                wact[:], wsrc[:, 0, 0:8],
                mybir.ActivationFunctionType.Exp, bias=0.0, scale=-1.0,
            )
            def warm_mms(k):
                # dep-free PE filler (reads the memset wsrc, writes a scratch
                # slot in the ghat bank, which has no real user until ~17us)
                wp = sqpsp.tile([P, P], FP32, tag="sqps", name="warm")
                for _ in range(k):
                    nc.tensor.matmul(
                        wp[:], wsrc[:], wsrc[:], start=True, stop=True,
                        perf_mode=mybir.MatmulPerfMode.DoubleRow,
                        skip_group_check=True,
                    )

            def p1_chunk(moff, c16, c8, j, lo, w):
                # columns [lo, lo+w) of this slice (global m = moff+lo..)
                ps = sqpsp.tile([P, mc], FP32, tag="sqps")
                nc.tensor.matmul(
                    ps[:, 0:w],
                    bf[:, :, j * P:(j + 1) * P],
                    af[:, :, moff + lo: moff + lo + w],
                    start=True, stop=True,
                    perf_mode=mybir.MatmulPerfMode.DoubleRow,
                )
                nc.scalar.activation(
                    c16[:, j, lo:lo + w], ps[:, 0:w],
                    mybir.ActivationFunctionType.Exp,
                    bias=0.0, scale=-1.0,
                )
                nc.vector.tensor_copy(
                    c8[:, j, lo:lo + w], c16[:, j, lo:lo + w],
                )

            def ghat_group(h, c16, i):
                gp = sqpsp.tile([P, 1], FP32, tag="sqps", name=f"gp_{h}_{i}")
                for j in range(kt):
                    nc.tensor.matmul(
                        gp[:], c16[:, j, i * P:(i + 1) * P], cv[:, j:j + 1],
                        start=(j == 0), stop=(j == kt - 1),
                        skip_group_check=True,
                    )
                gh = gsbp.tile([P, 1], FP32, tag="ghsb")
                nc.vector.tensor_copy(gh[:], gp[:])
                nc.sync.dma_start(ghat_o[h * mt + i, :], gh[:])

            # fused kk-pairs: one DMA per two 128-row k-slabs halves the
            # per-DMA engine issue cost (500ns floor) on the two queues
            def slab_tiles(r):
                return [slabp.tile([P, 2, 2, rb], FP8, tag="slab",
                                   name=f"sl_{r}_{kk2}")
                        for kk2 in range(kt2 // 2)]

            def slab_dma(eng, slabs_r, r, kk2):
                eng.dma_start(slabs_r[kk2][:],
                              th8[:, 2 * kk2:2 * kk2 + 2, :,
                                  r * rb:(r + 1) * rb])

            def slab_dmas(r):
                slabs = slab_tiles(r)
                for kk2 in range(kt2 // 2):
                    slab_dma(nc.sync if kk2 % 2 == 0 else nc.gpsimd,
                             slabs, r, kk2)
                return slabs

            def dr_mm(c8, tp, slabs, i, kk):
                nc.tensor.matmul(
                    tp[:],
                    c8[:, 2 * kk:2 * kk + 2, i * P:(i + 1) * P],
                    slabs[kk // 2][:, kk % 2],
                    start=(kk == 0), stop=(kk == kt2 - 1),
                    perf_mode=mybir.MatmulPerfMode.DoubleRow,
                    skip_group_check=True,
                )

            def square(h, gaccs, tp, r, i):
                sc = scrp.tile([P, rb], FP16, tag="scr",
                               name=f"sc_{h}_{r}_{i}")
                nc.scalar.activation(
                    sc[:], tp[:], mybir.ActivationFunctionType.Square,
                    accum_out=gaccs[i][:, r:r + 1],
                )

            # ---- emission order == scheduler priority (keep each engine's
            # priority stream temporally feasible: the wait queue is shallow,
            # so a long run of not-yet-ready instructions stalls the engine).
            # n0 is processed in progressive slices: only the narrow first
            # slice's exp production is PE-exposed; each later slice's
            # phase 1 hides as sprinkles inside the previous slice's GEMM.

            SL = (256, 512, 640, 640)          # slice widths, sum = n0
            nsl = len(SL)
            moffs = [sum(SL[:s]) for s in range(nsl)]
            c16s = [None] * nsl
            c8s = [None] * nsl

            def chunks_of(w):
                # full-width (cheap-overhead) chunks first, remainder after
                out = []
                lo = 0
                while lo < w:
                    out.append((lo, min(mc, w - lo)))
                    lo += mc
                return out

            def alloc_slice(s):
                c16s[s] = c16p.tile([P, kt, SL[s]], FP16, tag="c16",
                                    name=f"c16_{s}")
                c8s[s] = c8p.tile([P, kt, SL[s]], FP8, tag="c8",
                                  name=f"c8_{s}")

            def ghat_group_sl(s, i):
                gp = sqpsp.tile([P, 1], FP32, tag="sqps", name=f"gp_{s}_{i}")
                for j in range(kt):
                    nc.tensor.matmul(
                        gp[:], c16s[s][:, j, i * P:(i + 1) * P],
                        cv[:, j:j + 1],
                        start=(j == 0), stop=(j == kt - 1),
                        skip_group_check=True,
                    )
                gh = gsbp.tile([P, 1], FP32, tag="ghsb")
                nc.vector.tensor_copy(gh[:], gp[:])
                nc.sync.dma_start(ghat_o[moffs[s] // P + i, :], gh[:])

            # hand-scheduled prologue issuance: input pieces and the r0/r1/r2
            # slab streams interleaved so every tile lands just before its
            # first consumer. sync carries bf + even kk2-slabs; gpsimd
            # carries af/cv + odd kk2-slabs.
            slabs0 = slab_tiles(0)
            slabs1 = slab_tiles(1)
            slabs2 = slab_tiles(2)
            slab_cache = {(0, 2): slabs2}
            nc.sync.dma_start(bf[:, :, 0:512], b_feat[:, :, 0:512])
            nc.gpsimd.dma_start(af[:, :, 0:mc], a_feat[:, :, 0:mc])
            slab_dma(nc.sync, slabs0, 0, 0)
            slab_dma(nc.gpsimd, slabs0, 0, 1)
            nc.sync.dma_start(bf[:, :, 512:2048], b_feat[:, :, 512:2048])
            slab_dma(nc.gpsimd, slabs1, 1, 1)
            slab_dma(nc.sync, slabs1, 1, 0)
            nc.sync.dma_start(bf[:, :, 2048:n], b_feat[:, :, 2048:n])
            for kk2 in range(2, kt2 // 2, 2):
                slab_dma(nc.sync, slabs0, 0, kk2)
                slab_dma(nc.gpsimd, slabs0, 0, kk2 + 1)
                slab_dma(nc.sync, slabs1, 1, kk2)
                slab_dma(nc.gpsimd, slabs1, 1, kk2 + 1)
            for kk2 in range(0, kt2 // 2, 2):
                slab_dma(nc.sync, slabs2, 2, kk2)
                slab_dma(nc.gpsimd, slabs2, 2, kk2 + 1)
            nc.gpsimd.dma_start(af[:, :, mc:n0], a_feat[:, :, mc:n0])
            nc.gpsimd.dma_start(cv[:], cinv[:])
            # phase 1 slice 0, interleaved with its own GEMM r0+r1: the exp
            # chain paces ACT at ~610ns/pair; feeding PE the two-pairs-ago
            # c8 planes (4 dr_mms per pair, ~530ns) keeps PE busy through
            # the whole slice-0 ramp instead of stalling on ACT. The LAG
            # keeps PE's in-order stream off the mm->exp->convert latency.
            alloc_slice(0)
            w0 = SL[0]
            mt0 = SL[0] // P
            LAG = 2
            gaccs0 = [gsbp.tile([P, nrb], FP32, tag="gacc",
                                name=f"gacc_0_{i}") for i in range(mt0)]
            tA = {i: tpsp.tile([P, rb], FP32, tag="tps", name=f"tp_a_{i}")
                  for i in range(mt0)}
            tB = {i: tpsp.tile([P, rb], FP32, tag="tps", name=f"tp_b_{i}")
                  for i in range(mt0)}
            for jp in range(kt2 + LAG):
                if jp == 0 and warm:
                    warm_mms(warm)   # dep-free head filler + p-state ramp
                if jp < kt2:
                    ps = sqpsp.tile([P, mc], FP32, tag="sqps")
                    for u in range(2):
                        nc.tensor.matmul(
                            ps[:, u * w0:(u + 1) * w0],
                            bf[:, :, (2 * jp + u) * P:(2 * jp + u + 1) * P],
                            af[:, :, moffs[0]:moffs[0] + w0],
                            start=True, stop=True,
                            perf_mode=mybir.MatmulPerfMode.DoubleRow,
                        )
                    nc.scalar.activation(
                        c16s[0][:, 2 * jp:2 * jp + 2, :], ps[:],
                        mybir.ActivationFunctionType.Exp,
                        bias=0.0, scale=-1.0,
                    )
                    nc.vector.tensor_copy(
                        c8s[0][:, 2 * jp:2 * jp + 2, :],
                        c16s[0][:, 2 * jp:2 * jp + 2, :],
                    )
                kk = jp - LAG
                if kk >= 0:
                    for i in range(mt0):
                        dr_mm(c8s[0], tA[i], slabs0, i, kk)
                    for i in range(mt0):
                        dr_mm(c8s[0], tB[i], slabs1, i, kk)
            for i in range(mt0):
                square(0, gaccs0, tA[i], 0, i)
            for i in range(mt0):
                square(0, gaccs0, tB[i], 1, i)

            def emit_gemm_sprinkled(s, gaccs_pre):
                """GEMM for slice s; sprinkle phase-1 of slice s+1."""
                c8 = c8s[s]
                mts = SL[s] // P
                last_sl = s == nsl - 1
                if gaccs_pre is None:
                    gaccs = [gsbp.tile([P, nrb + (1 if last_sl else 0)],
                                       FP32, tag="gacc",
                                       name=f"gacc_{s}_{i}")
                             for i in range(mts)]
                else:
                    gaccs = gaccs_pre
                ns = s + 1 if s + 1 < nsl else None
                if ns is not None:
                    alloc_slice(ns)
                    p1_list = [(j, lo, w) for j in range(kt)
                               for lo, w in chunks_of(SL[ns])]
                r_lo = 2 if s == 0 else 0
                ngrp = (nrb - r_lo) * mts
                g = 0
                cc = 0
                # all ghat groups up front (c16/cv are long since ready, PE
                # cost ~0); consecutive emission keeps the sqps rotation free
                # of alloc cycles against the sprinkled exps
                for i in range(mts):
                    ghat_group_sl(s, i)
                for r in range(r_lo, nrb):
                    slabs = slab_cache.pop((s, r), None) or slab_dmas(r)
                    for i in range(mts):
                        if last_sl and r == nrb - 1 and i == mts - 1:
                            # final tile of the kernel: split its last
                            # r-block 384+128 so the tail-exposed square
                            # (after the very last matmul) is narrow
                            for lo, w, col in ((0, 384, r), (384, 128, nrb)):
                                tp = tpsp.tile([P, w], FP32, tag="tps",
                                               name=f"tp_{s}_{r}_{i}_{lo}")
                                for kk in range(kt2):
                                    nc.tensor.matmul(
                                        tp[:],
                                        c8[:, 2 * kk:2 * kk + 2,
                                           i * P:(i + 1) * P],
                                        slabs[kk // 2][:, kk % 2, :,
                                                       lo:lo + w],
                                        start=(kk == 0),
                                        stop=(kk == kt2 - 1),
                                        perf_mode=(
                                            mybir.MatmulPerfMode.DoubleRow),
                                        skip_group_check=True,
                                    )
                                sc = scrp.tile([P, w], FP16, tag="scr",
                                               name=f"sce_{lo}")
                                nc.scalar.activation(
                                    sc[:], tp[:],
                                    mybir.ActivationFunctionType.Square,
                                    accum_out=gaccs[i][:, col:col + 1],
                                )
                            g += 1
                            continue
                        tp = tpsp.tile([P, rb], FP32, tag="tps",
                                       name=f"tp_{s}_{r}_{i}")
                        for kk in range(kt2):
                            dr_mm(c8, tp, slabs, i, kk)
                        square(s, gaccs, tp, r, i)
                        g += 1
                        if ns is not None:
                            # start sprinkling `lead` groups in: the slice
                            # transition already has an ACT backlog (squares
                            # of the previous slice's last r-block), so the
                            # first groups run sprinkle-free
                            want = (max(0, g - lead) * len(p1_list)
                                    // (ngrp - lead))
                            while cc < want:
                                p1_chunk(moffs[ns], c16s[ns], c8s[ns],
                                         *p1_list[cc])
                                cc += 1
                if ns is not None:
                    while cc < len(p1_list):
                        p1_chunk(moffs[ns], c16s[ns], c8s[ns], *p1_list[cc])
                        cc += 1
                nc_cols = nrb + (1 if last_sl else 0)
                for i in range(mts):
                    # the very last tile's DMA goes out on ACT: same-engine
                    # ordering after its square skips a cross-engine sem hop
                    eng = (nc.scalar if last_sl and i == mts - 1
                           else nc.sync)
                    eng.dma_start(
                        sumt2_o[moffs[s] // P + i, :, 0:nc_cols], gaccs[i][:])

            emit_gemm_sprinkled(0, gaccs0)
            for s in range(1, nsl):
                emit_gemm_sprinkled(s, None)

    nc.compile()
    return nc


def _features_for_q(x0s, x, inv_l_q, lLmb0_q, fk=20):
    """Host prep: hi/lo-split fp8e4 feature row pairs (DoubleRow layout) so
    the PE computes sq_mod[n, m] = ||a_m - b_n||^2 - lLmb0 - ln(S_C0) with
    ~8-bit products and 3-level sq rows (abs err ~2e-3)."""
    f32 = np.float32

    def hilo8(v, levels=2):
        outs = []
        r = v.astype(f32).copy()
        for _ in range(levels):
            h = r.astype(F8NP)
            outs.append(h)
            r = r - h.astype(f32)
        return outs

    a = (x0s * inv_l_q).astype(f32)            # [n0, d]
    b = (x * inv_l_q).astype(f32)              # [n, d]
    sqa = (a * a).sum(-1, dtype=f32) - f32(lLmb0_q) - f32(LN_S_C0)
    sqb = (b * b).sum(-1, dtype=f32)
    m2a = (-2.0 * a).astype(f32)
    Ah, Al = hilo8(m2a)
    Bh, Bl = hilo8(b)
    sqa3 = hilo8(sqa, 3)
    sqb3 = hilo8(sqb, 3)
    d = a.shape[1]
    n0, n = a.shape[0], b.shape[0]
    ones_a = np.ones(n0, F8NP)
    ones_b = np.ones(n, F8NP)
    pairs = []                                  # (a_row [n0], b_row [n])
    for dd in range(d):
        pairs.append((Ah[:, dd], Bh[:, dd]))
        pairs.append((Ah[:, dd], Bl[:, dd]))
        pairs.append((Al[:, dd], Bh[:, dd]))
        pairs.append((Al[:, dd], Bl[:, dd]))
    for lev in sqa3:
        pairs.append((lev, ones_b))
    for lev in sqb3:
        pairs.append((ones_a, lev))
    assert len(pairs) <= 2 * fk
    af = np.zeros((fk, 2, n0), F8NP)
    bf = np.zeros((fk, 2, n), F8NP)
    for v, (ar, br) in enumerate(pairs):
        af[v // 2, v % 2] = ar
        bf[v // 2, v % 2] = br
    return af, bf


def _th_scale(th_q):
    """Power-of-two scale putting max |Th| into (96, 192] for fp8e4."""
    m = float(np.abs(th_q).max())
    if m == 0.0:
        return np.float32(1.0)
    return np.float32(2.0 ** np.floor(np.log2(192.0 / m)))


def prep_core_inputs(inputs, q, fk=32):
    """Per-core (per-component) input map for the device kernel."""
    f16, f32 = np.float16, np.float32
    x0 = np.asarray(inputs["x0"], f32)
    x = np.asarray(inputs["x"], f32)
    x_min = np.asarray(inputs["x_min"], f32)
    x_max = np.asarray(inputs["x_max"], f32)
    lLmb = np.asarray(inputs["lLmb"], f32)
    lLmb0 = np.asarray(inputs["lLmb0"], f32)
    x0s = (x0 - x_min) / (x_max - x_min)
    inv_l = np.exp(-0.5 * lLmb[q]).astype(f32)
    af, bf = _features_for_q(x0s, x, inv_l, lLmb0[q], fk=20)
    cinv = np.asarray(inputs["CinvM"], f32)[q].astype(f16)
    n = cinv.shape[0]
    cinv_t = np.ascontiguousarray(cinv.reshape(n // P, P).T)   # [128, kt]
    th_q = np.asarray(inputs["Th"], f32)[q]
    s_th = _th_scale(th_q)
    # [128, kt2, 2, n]: th8[p, kk, i, c] = s_th * Th[kk*256 + i*128 + p, c]
    th8 = np.ascontiguousarray(
        (th_q.reshape(n // 256, 2, P, n) * s_th).transpose(2, 0, 1, 3)
    ).astype(F8NP)
    return {"a_feat": af, "b_feat": bf, "th8": th8, "cinv": cinv_t}


def finish_host(inputs, ghat_all, sumt2_all):
    """Final tiny [q,n0] -> [p,n0] projection, fp32 on host (mirrors reference)."""
    f32 = np.float32
    lLmb0 = np.asarray(inputs["lLmb0"], f32)
    lnug = np.asarray(inputs["lnugGPs"], f32)
    lsig = np.asarray(inputs["lsigma2s"], f32)
    phi = np.asarray(inputs["phi"], f32)
    ystd = np.asarray(inputs["ystd"], f32)
    ymean = np.asarray(inputs["ymean"], f32)

    c00 = (np.exp(lLmb0) * (1.0 + np.exp(lnug))).astype(f32)[:, None]
    gvar = c00 - sumt2_all                        # [q, n0]
    sig = np.exp(lsig).astype(f32)                # [p]
    psi = (phi * np.sqrt(sig)[:, None]).astype(f32)
    predmean = (psi @ ghat_all).astype(f32)       # [p, n0]
    confvar = (gvar.T @ (psi ** 2).T).astype(f32)  # [n0, p]
    predvar = confvar + sig
    ypred = (predmean * ystd + ymean).astype(f32)
    yconfvar = (confvar.T * ystd ** 2).astype(f32)
    ypredvar = (predvar.T * ystd ** 2).astype(f32)
    return ypred, ypredvar, yconfvar


_NC_CACHE = {}
LAST_RESULTS = None


def kernel(**inputs):
    from concourse.bass_utils import run_bass_kernel_spmd

    global LAST_RESULTS
    q_n = Q_FULL
    n0 = N0_FULL

    if "nc" not in _NC_CACHE:
        _NC_CACHE["nc"] = build_nc()
    nc = _NC_CACHE["nc"]

    th_f32 = np.asarray(inputs["Th"], np.float32)
    s_ths = [_th_scale(th_f32[q]) for q in range(q_n)]
    in_maps = [prep_core_inputs(inputs, q) for q in range(q_n)]
    core_ids = list(range(q_n))
    res = run_bass_kernel_spmd(
        nc, in_maps, core_ids,
        trace=bool(os.environ.get("LCGP_TRACE")),
    )
    LAST_RESULTS = res

    ghat_all = np.zeros((q_n, n0), np.float32)
    sumt2_all = np.zeros((q_n, n0), np.float32)
    for q in range(q_n):
        ghat_all[q] = np.asarray(res.results[q]["ghat"]).reshape(n0) / S_C0
        raw = np.asarray(res.results[q]["sumt2"], np.float32)  # [16,128,9]
        s2 = raw[:, :, :8].sum(-1, dtype=np.float32)
        s2[11:] += raw[11:, :, 8]     # final-slice tiles carry a split chunk
        sumt2_all[q] = s2.reshape(n0) / (S_C0 * s_ths[q]) ** 2

    return finish_host(inputs, ghat_all, sumt2_all)

